# revision 32
# baseline (speedup 1.0000x reference)
"""Trainium2 Bass kernel for nn_AMLNeuralNetwork3D (dense_mlp).

Strategy v7: layers 1 and 2 are ROW-parallel (each core contracts over
its own 1024 input features, producing a [8192, batch] partial); layer 3
is column-parallel.

  local (per-core feats, DVE)            -> h0_own   [1024, b] SBUF
  L1rp: partial1 = W1[:, own].T-contract -> part1    [8192, b] DRAM
  ReduceScatter(add, bf16)               -> y1_own   [1024, b]
  relu + b1 (DVE)                        -> h1_own   [1024, b] SBUF
  L2rp: partial2 = W2[:, own].T-contract -> part2    [8192, b] DRAM
  AllReduce(add, bf16)                   -> y2 full  [8192, b] everywhere
  L3col: relu(y2+b2) per-tile on DVE at load, W3 column-split,
         relu+b3 -> out slice [1024, b] fp32

Why: the PE starts real matmuls ~12us into the kernel with ZERO
preceding communication (launch barrier and launch skew hide behind L1
compute), the L1->L2 transition moves only 7MB/core (RS) instead of a
14MB AllReduce, and there is no tail collective.  Per-chunk (batch 512)
pipelining overlaps every collective with a full chunk of compute.

Engine/ring discipline (critical): the sync ring carries ONLY DMAs whose
data is ready when the ring reaches them (x, weights) plus
collective-fed streams explicitly gated (add_dep_helper) behind the
previous layer's last matmuls -- an ungated collective-gated DMA gets
hoisted by the Tile scheduler into the previous layer's stream and
head-of-line blocks the whole 16-queue DMA pool for ~50us.  The scalar
ring carries PE-paced PSUM evictions and output writes; the vector
engine + gpsimd ring carry everything else that waits on collectives.

Compute in bf16 (fp32 PSUM accumulation); partial sums cross cores in
bf16 (adds ~1e-3 rel err; total ~7e-3 vs the 2e-2 gate).
"""

import sys

if "/opt/trn_rl_repo" not in sys.path:
    sys.path.insert(0, "/opt/trn_rl_repo")

import numpy as np
import ml_dtypes

N_CORES = 8
G = 8192          # genes / features
B = 1024          # batch
L = 4             # levels
GS = G // N_CORES # per-core feature slice (1024)
NB = 512          # batch chunk (one PSUM bank at fp32)
NCHUNK = B // NB  # 2
GT = GS // 128    # gene tiles per core slice (8)
KT = G // 128     # contraction tiles (64)

BF16 = ml_dtypes.bfloat16

_compiled = {}


def _build_graph():
    from concourse import bacc, tile
    from concourse.tile_rust import add_dep_helper
    import concourse.mybir as mybir

    fp32 = mybir.dt.float32
    bf16 = mybir.dt.bfloat16
    Relu = mybir.ActivationFunctionType.Relu
    Copy = mybir.ActivationFunctionType.Copy
    mult = mybir.AluOpType.mult
    add = mybir.AluOpType.add
    amax = mybir.AluOpType.max

    nc = bacc.Bacc(None, target_bir_lowering=False, num_devices=N_CORES)

    # ---- parameters (per-core shards; same graph on all cores) ----
    x_p = nc.declare_dram_parameter("x", [L, GS, B], bf16, isOutput=False)
    # per-feature scalars: cols 0..3 = W_local, 4 = b_local, 5..7 = b1..b3
    scal_p = nc.declare_dram_parameter("scal", [GS, 8], fp32, isOutput=False)
    # row-parallel weights: [own 1024 in-feats, 8192 out-feats plain order]
    w1rp_p = nc.declare_dram_parameter("w1rp", [GS, G], bf16, isOutput=False)
    w2rp_p = nc.declare_dram_parameter("w2rp", [GS, G], bf16, isOutput=False)
    # L3: [8192 in-feats plain order, own 1024 out-feats]
    w3t_p = nc.declare_dram_parameter("w3t", [G, GS], bf16, isOutput=False)
    # full b2: [128 rows-in-ktile, 64 ktiles]
    b2p_p = nc.declare_dram_parameter("b2p", [128, KT], fp32, isOutput=False)
    out_p = nc.declare_dram_parameter("out", [GS, B], fp32, isOutput=True)

    rg = [list(range(N_CORES))]

    with tile.TileContext(nc) as tc:
        with (
            tc.tile_pool(name="dram", bufs=1, space="DRAM") as dram,
            tc.tile_pool(name="scal", bufs=GT) as spool,
            tc.tile_pool(name="xin", bufs=16) as xpool,
            tc.tile_pool(name="loc", bufs=10) as lpool,
            tc.tile_pool(name="h0", bufs=16) as h0pool,
            tc.tile_pool(name="h1", bufs=16) as h1pool,
            tc.tile_pool(name="hin", bufs=40) as hpool,
            tc.tile_pool(name="wblk", bufs=24) as wpool,
            tc.tile_pool(name="evic", bufs=36) as epool,
            tc.tile_pool(name="hout", bufs=8) as opool,
            tc.tile_pool(name="psum", bufs=8, space="PSUM") as ppool,
        ):
            part1 = [
                dram.tile([G, NB], bf16, name=f"p1_{j}", tag=f"p1_{j}")
                for j in range(NCHUNK)
            ]
            rs1o = [
                dram.tile([GS, NB], bf16, name=f"r1_{j}", tag=f"r1_{j}")
                for j in range(NCHUNK)
            ]
            part2 = [
                dram.tile([G, NB], bf16, name=f"p2_{j}", tag=f"p2_{j}")
                for j in range(NCHUNK)
            ]
            arout2 = [
                dram.tile([G, NB], bf16, name=f"a2_{j}", tag=f"a2_{j}",
                          addr_space="Shared")
                for j in range(NCHUNK)
            ]

            # per-feature scalar tiles, persistent
            sc = []
            for gt in range(GT):
                s = spool.tile([128, 8], fp32, name=f"sc{gt}", tag="sc")
                nc.sync.dma_start(s[:], scal_p[gt * 128 : (gt + 1) * 128, :])
                sc.append(s)
            b2p = spool.tile([128, KT], fp32, name="b2p", tag="b2p")
            nc.sync.dma_start(b2p[:], b2p_p[:, :])

            h0 = [[None] * GT for _ in range(NCHUNK)]
            h1 = [[None] * GT for _ in range(NCHUNK)]

            def local_layer(j):
                # entirely on the vector engine (+ sync ring for x) so the
                # scalar ring stays free for PE-paced evictions.
                for gt in range(GT):
                    xt = []
                    for l in range(L):
                        t = xpool.tile([128, NB], bf16, name=f"x{j}_{gt}_{l}", tag="x")
                        nc.sync.dma_start(
                            t[:],
                            x_p[l, gt * 128 : (gt + 1) * 128, j * NB : (j + 1) * NB],
                        )
                        xt.append(t)
                    acc = lpool.tile([128, NB], bf16, name=f"a{j}_{gt}_0", tag="acc")
                    nc.vector.tensor_scalar(
                        acc[:], xt[0][:], sc[gt][:, 0:1], None, mult
                    )
                    for l in range(1, L):
                        acc2 = lpool.tile(
                            [128, NB], bf16, name=f"a{j}_{gt}_{l}", tag="acc"
                        )
                        nc.vector.scalar_tensor_tensor(
                            acc2[:], xt[l][:], sc[gt][:, l : l + 1], acc[:], mult, add
                        )
                        acc = acc2
                    t = h0pool.tile([128, NB], bf16, name=f"h0_{j}_{gt}", tag="h0")
                    nc.vector.tensor_scalar(
                        t[:], acc[:], sc[gt][:, 4:5], 0.0, add, amax
                    )
                    h0[j][gt] = t

            def load_w_og(wp, lbl, j, og):
                wb = []
                for k in range(GT):
                    w = wpool.tile([128, 1024], bf16, name=f"{lbl}_{j}_{og}_{k}",
                                   tag="wblk")
                    nc.sync.dma_start(
                        w[:],
                        wp[k * 128 : (k + 1) * 128, og * 1024 : (og + 1) * 1024],
                    )
                    wb.append(w)
                return wb

            def rp_layer(wp, lbl, rhs, target, j, preloaded=None):
                # row-parallel dense layer: partial[o, b] over own 1024 input
                # features.  og = 1024-wide output group; two half-og PSUM
                # groups of 4 banks so evictions overlap the next group's
                # matmuls.  Returns (last_mm, og6_mm) markers.
                last_mm = None
                og6_mm = None
                for og in range(GT):
                    if preloaded is not None and og in preloaded:
                        wb = preloaded[og]
                    else:
                        wb = load_w_og(wp, lbl, j, og)
                    for half in range(2):
                        ps = [
                            ppool.tile([128, NB], fp32,
                                       name=f"ps{lbl}_{j}_{og}_{half}_{oo}",
                                       tag="ps")
                            for oo in range(4)
                        ]
                        for k in range(GT):
                            for oo in range(4):
                                ocol = half * 4 + oo
                                last_mm = nc.tensor.matmul(
                                    ps[oo][:],
                                    wb[k][:, ocol * 128 : (ocol + 1) * 128],
                                    rhs[k][:],
                                    start=(k == 0),
                                    stop=(k == GT - 1),
                                )
                        if og == 6 and half == 0 and og6_mm is None:
                            og6_mm = last_mm
                        for oo in range(4):
                            t = epool.tile([128, NB], bf16,
                                           name=f"ev{lbl}_{j}_{og}_{half}_{oo}",
                                           tag="ev")
                            nc.scalar.activation(t[:], ps[oo][:], Copy)
                            row = og * 1024 + half * 512 + oo * 128
                            # part writes ride the scalar ring: they pace
                            # with the evictions, never blocking prefetch
                            nc.scalar.dma_start(
                                target[row : row + 128, :], t[:]
                            )
                return last_mm, og6_mm

            def rs1(j):
                nc.gpsimd.collective_compute(
                    "ReduceScatter", add, replica_groups=rg,
                    ins=[part1[j][:].opt()],
                    outs=[rs1o[j][:].opt()],
                )

            def relu1(j):
                # y1_own -> relu(y1 + b1) -> h1 SBUF tiles feeding L2rp.
                # gpsimd ring for the reads (naturally ordered after the RS
                # on that queue), DVE for the relu.
                for gt in range(GT):
                    tin = hpool.tile([128, NB], bf16, name=f"ri_{j}_{gt}",
                                     tag="hin")
                    nc.gpsimd.dma_start(
                        tin[:], rs1o[j][gt * 128 : (gt + 1) * 128, :]
                    )
                    t = h1pool.tile([128, NB], bf16, name=f"h1_{j}_{gt}",
                                    tag="h1")
                    nc.vector.tensor_scalar(
                        t[:], tin[:], sc[gt][:, 5:6], 0.0, add, amax
                    )
                    h1[j][gt] = t

            def ar2(j):
                nc.gpsimd.collective_compute(
                    "AllReduce", add, replica_groups=rg,
                    ins=[part2[j][:].opt()],
                    outs=[arout2[j][:].opt()],
                )

            def dense3(j, gate):
                # column-parallel L3: input = relu(y2 + b2) applied per-tile
                # on the vector engine; output slice -> out_p.
                # `gate` keeps the AllReduce-fed raw DMAs placed after the
                # previous layer's ring traffic (see module docstring).
                last_mm = None
                g48_mm = None
                ps = [
                    ppool.tile([128, NB], fp32, name=f"ps3_{j}_{o}", tag="ps")
                    for o in range(GT)
                ]
                for g in range(KT):
                    raw = hpool.tile([128, NB], bf16, name=f"r3_{j}_{g}",
                                     tag="hin")
                    hdma = nc.sync.dma_start(
                        raw[:], arout2[j][g * 128 : (g + 1) * 128, :]
                    )
                    if gate is not None:
                        add_dep_helper(
                            getattr(hdma, "ins", hdma),
                            getattr(gate, "ins", gate),
                            reason="keep AllReduce-fed DMAs behind the "
                                   "previous layer's ring traffic",
                        )
                    ht = hpool.tile([128, NB], bf16, name=f"h3_{j}_{g}",
                                    tag="hin")
                    nc.vector.tensor_scalar(
                        ht[:], raw[:], b2p[:, g : g + 1], 0.0, add, amax
                    )
                    wb = wpool.tile([128, GS], bf16, name=f"w3_{j}_{g}",
                                    tag="wblk")
                    nc.sync.dma_start(wb[:], w3t_p[g * 128 : (g + 1) * 128, :])
                    for o in range(GT):
                        last_mm = nc.tensor.matmul(
                            ps[o][:],
                            wb[:, o * 128 : (o + 1) * 128],
                            ht[:],
                            start=(g == 0),
                            stop=(g == KT - 1),
                        )
                    if g == 48:
                        g48_mm = last_mm
                for o in range(GT):
                    ot = opool.tile([128, NB], fp32, name=f"o3_{j}_{o}",
                                    tag="outp")
                    nc.scalar.activation(ot[:], ps[o][:], Relu, bias=sc[o][:, 7:8])
                    nc.scalar.dma_start(
                        out_p[o * 128 : (o + 1) * 128, j * NB : (j + 1) * NB],
                        ot[:],
                    )
                return last_mm, g48_mm

            # emission order = desired overlap order.  Preload L1c0's first
            # weight group ahead of the x stream so the PE starts ~12us in.
            pre = {0: load_w_og(w1rp_p, "w1", 0, 0)}
            local_layer(0)
            rp_layer(w1rp_p, "w1", h0[0], part1[0], 0, preloaded=pre)
            local_layer(1)
            rs1(0)
            relu1(0)
            rp_layer(w1rp_p, "w1", h0[1], part1[1], 1)
            rs1(1)
            relu1(1)
            rp_layer(w2rp_p, "w2", h1[0], part2[0], 0)
            ar2(0)
            _, m_l2c1 = rp_layer(w2rp_p, "w2", h1[1], part2[1], 1)
            ar2(1)
            _, m_d3c0 = dense3(0, gate=m_l2c1)
            dense3(1, gate=m_d3c0)

    nc.compile()
    return nc


def _get_nc():
    if "nc" not in _compiled:
        _compiled["nc"] = _build_graph()
    return _compiled["nc"]


def kernel(x, W_local, b_local, W1, b1, W2, b2, W3, b3):
    from concourse.bass_utils import run_bass_kernel_spmd

    nc = _get_nc()

    x = np.asarray(x)
    b2p = np.ascontiguousarray(
        np.asarray(b2).reshape(KT, 128).T
    ).astype(np.float32)
    in_maps = []
    for r in range(N_CORES):
        sl = slice(r * GS, (r + 1) * GS)
        x_r = x[:, :, sl].transpose(0, 2, 1).astype(BF16)
        scal_r = np.concatenate(
            [
                np.asarray(W_local)[sl, :],
                np.asarray(b_local)[sl, None],
                np.asarray(b1)[sl, None],
                np.asarray(b2)[sl, None],
                np.asarray(b3)[sl, None],
            ],
            axis=1,
        ).astype(np.float32)
        in_maps.append(
            {
                "x": x_r,
                "scal": np.ascontiguousarray(scal_r),
                "b2p": b2p,
                # [own 1024 in-feats, 8192 plain out-feats]
                "w1rp": np.ascontiguousarray(np.asarray(W1)[:, sl].T).astype(BF16),
                "w2rp": np.ascontiguousarray(np.asarray(W2)[:, sl].T).astype(BF16),
                # [8192 plain in-feats, own 1024 out-feats]
                "w3t": np.asarray(W3)[sl, :].T.astype(BF16),
            }
        )

    res = run_bass_kernel_spmd(nc, in_maps, core_ids=list(range(N_CORES)))

    out = np.empty((B, G), np.float32)
    for r in range(N_CORES):
        out[:, r * GS : (r + 1) * GS] = res.results[r]["out"].T
    return out


# revision 33
# speedup vs baseline: 1.1074x; 1.1074x over previous
"""Trainium2 Bass kernel for nn_AMLNeuralNetwork3D (dense_mlp).

Strategy v8: layer 1 is ROW-parallel (each core contracts over its own
1024 input features, producing a [8192, batch] partial), so the PE
starts real matmuls ~12us into the kernel with ZERO preceding
communication -- the launch barrier and launch skew hide behind L1
compute.  Partials are summed with bf16 AllReduce ops (chunk 0 as two
feature halves for early first delivery under launch skew; chunk 1 as
one op for a shorter serial CC chain); every core then has the full
pre-relu y1, and relu+b1 is applied per-tile on the otherwise-idle
vector engine as L2 streams its input.  Layers 2 and 3 are
column-parallel: L2 consumes the AllReduce halves (W2 rows permuted
into half-major order), AllGather, L3 consumes the full gather and
writes the per-core output slice.

Engine/ring discipline: the sync ring carries only monotonically-ready
DMAs (x, weights, gathered streams in consumption order), the scalar
ring carries PE-paced PSUM evictions and output writes, the vector
engine + gpsimd ring carry everything gated on collectives.  The weight
pool is deep (40 blocks, 10MB) so the PE coasts on prefetched weights
through the windows where a collective-gated DMA head-of-line blocks
the sync ring (which in turn leaves DRAM bandwidth to the collective).

Compute in bf16 (fp32 PSUM accumulation); partial sums cross cores in
bf16 (total rel err ~6e-3 vs the 2e-2 gate).
"""

import sys

if "/opt/trn_rl_repo" not in sys.path:
    sys.path.insert(0, "/opt/trn_rl_repo")

import numpy as np
import ml_dtypes

N_CORES = 8
G = 8192          # genes / features
B = 1024          # batch
L = 4             # levels
GS = G // N_CORES # per-core feature slice (1024)
NB = 512          # batch chunk (one PSUM bank at fp32)
NCHUNK = B // NB  # 2
GT = GS // 128    # gene tiles per core slice (8)
KT = G // 128     # contraction tiles (64)

BF16 = ml_dtypes.bfloat16

_compiled = {}

# half-major feature order: [core0 f0:512, core1 f1024:1536, ...] then the
# second halves.  This is the row order of the L1 partials / AllReduce
# output consumed by L2.
_PERM_HALVES = np.concatenate(
    [np.arange(r * 1024, r * 1024 + 512) for r in range(8)]
    + [np.arange(r * 1024 + 512, (r + 1) * 1024) for r in range(8)]
)


def _build_graph():
    from concourse import bacc, tile
    import concourse.mybir as mybir

    fp32 = mybir.dt.float32
    bf16 = mybir.dt.bfloat16
    Relu = mybir.ActivationFunctionType.Relu
    Copy = mybir.ActivationFunctionType.Copy
    mult = mybir.AluOpType.mult
    add = mybir.AluOpType.add
    amax = mybir.AluOpType.max
    bypass = mybir.AluOpType.bypass

    nc = bacc.Bacc(None, target_bir_lowering=False, num_devices=N_CORES)

    # ---- parameters (per-core shards; same graph on all cores) ----
    x_p = nc.declare_dram_parameter("x", [L, GS, B], bf16, isOutput=False)
    # per-feature scalars: cols 0..3 = W_local, 4 = b_local, 5..7 = b1..b3
    scal_p = nc.declare_dram_parameter("scal", [GS, 8], fp32, isOutput=False)
    # full b1 in half-major order: [128 rows-in-ktile, 64 ktiles]
    b1p_p = nc.declare_dram_parameter("b1p", [128, KT], fp32, isOutput=False)
    # L1 row-parallel weights: [own 1024 in-feats, 8192 half-major out-feats]
    w1rp_p = nc.declare_dram_parameter("w1rp", [GS, G], bf16, isOutput=False)
    # L2: [8192 half-major in-feats, own 1024 out-feats]
    w2t_p = nc.declare_dram_parameter("w2t", [G, GS], bf16, isOutput=False)
    # L3: [8192 plain in-feats, own 1024 out-feats]
    w3t_p = nc.declare_dram_parameter("w3t", [G, GS], bf16, isOutput=False)
    out_p = nc.declare_dram_parameter("out", [GS, B], fp32, isOutput=True)

    rg = [list(range(N_CORES))]

    with tile.TileContext(nc) as tc:
        with (
            tc.tile_pool(name="dram", bufs=1, space="DRAM") as dram,
            tc.tile_pool(name="scal", bufs=GT) as spool,
            tc.tile_pool(name="xin", bufs=16) as xpool,
            tc.tile_pool(name="loc", bufs=10) as lpool,
            tc.tile_pool(name="h0", bufs=16) as h0pool,
            tc.tile_pool(name="hin", bufs=28) as hpool,
            tc.tile_pool(name="wblk", bufs=40) as wpool,
            tc.tile_pool(name="evic", bufs=30) as epool,
            tc.tile_pool(name="hout", bufs=8) as opool,
            tc.tile_pool(name="psum", bufs=8, space="PSUM") as ppool,
        ):
            # L1 partials: chunk 0 in half-major halves, chunk 1 whole
            part0 = [
                dram.tile([G // 2, NB], bf16, name=f"part_0_{a}",
                          tag=f"part_0_{a}")
                for a in range(2)
            ]
            part1 = dram.tile([G, NB], bf16, name="part_1", tag="part_1")
            arout0 = [
                dram.tile([G // 2, NB], bf16, name=f"ar_0_{a}",
                          tag=f"ar_0_{a}", addr_space="Shared")
                for a in range(2)
            ]
            arout1 = dram.tile([G, NB], bf16, name="ar_1", tag="ar_1",
                               addr_space="Shared")
            # L2 -> L3 transition
            slc2 = [
                dram.tile([GS, NB], bf16, name=f"slc2_{j}", tag=f"slc2_{j}")
                for j in range(NCHUNK)
            ]
            gath2 = [
                dram.tile([G, NB], bf16, name=f"g2_{j}", tag=f"g2_{j}",
                          addr_space="Shared")
                for j in range(NCHUNK)
            ]

            # per-feature scalar tiles, persistent
            sc = []
            for gt in range(GT):
                s = spool.tile([128, 8], fp32, name=f"sc{gt}", tag="sc")
                nc.sync.dma_start(s[:], scal_p[gt * 128 : (gt + 1) * 128, :])
                sc.append(s)
            b1p = spool.tile([128, KT], fp32, name="b1p", tag="b1p")
            nc.sync.dma_start(b1p[:], b1p_p[:, :])

            h0 = [[None] * GT for _ in range(NCHUNK)]

            def local_layer(j):
                # entirely on the vector engine (+ sync ring for x) so the
                # scalar ring stays free for PE-paced evictions.
                for gt in range(GT):
                    xt = []
                    for l in range(L):
                        t = xpool.tile([128, NB], bf16, name=f"x{j}_{gt}_{l}", tag="x")
                        nc.sync.dma_start(
                            t[:],
                            x_p[l, gt * 128 : (gt + 1) * 128, j * NB : (j + 1) * NB],
                        )
                        xt.append(t)
                    acc = lpool.tile([128, NB], bf16, name=f"a{j}_{gt}_0", tag="acc")
                    nc.vector.tensor_scalar(
                        acc[:], xt[0][:], sc[gt][:, 0:1], None, mult
                    )
                    for l in range(1, L):
                        acc2 = lpool.tile(
                            [128, NB], bf16, name=f"a{j}_{gt}_{l}", tag="acc"
                        )
                        nc.vector.scalar_tensor_tensor(
                            acc2[:], xt[l][:], sc[gt][:, l : l + 1], acc[:], mult, add
                        )
                        acc = acc2
                    t = h0pool.tile([128, NB], bf16, name=f"h0_{j}_{gt}", tag="h0")
                    nc.vector.tensor_scalar(
                        t[:], acc[:], sc[gt][:, 4:5], 0.0, add, amax
                    )
                    h0[j][gt] = t

            def load_w1_og(j, og):
                wb = []
                for k in range(GT):
                    w = wpool.tile([128, 1024], bf16, name=f"w1_{j}_{og}_{k}",
                                   tag="wblk")
                    nc.sync.dma_start(
                        w[:],
                        w1rp_p[k * 128 : (k + 1) * 128,
                               og * 1024 : (og + 1) * 1024],
                    )
                    wb.append(w)
                return wb

            def l1rp(j, preloaded=None):
                # row-parallel L1: partial[o, b] over own 1024 input feats.
                # og = 1024-wide output group; two half-og PSUM groups of 4
                # banks each so evictions overlap the next group's matmuls.
                for og in range(GT):
                    if preloaded is not None and og in preloaded:
                        wb = preloaded[og]
                    else:
                        wb = load_w1_og(j, og)
                    for half in range(2):
                        ps = [
                            ppool.tile([128, NB], fp32,
                                       name=f"ps1_{j}_{og}_{half}_{oo}", tag="ps")
                            for oo in range(4)
                        ]
                        for k in range(GT):
                            for oo in range(4):
                                ocol = half * 4 + oo
                                nc.tensor.matmul(
                                    ps[oo][:],
                                    wb[k][:, ocol * 128 : (ocol + 1) * 128],
                                    h0[j][k][:],
                                    start=(k == 0),
                                    stop=(k == GT - 1),
                                )
                        for oo in range(4):
                            t = epool.tile([128, NB], bf16,
                                           name=f"ev_{j}_{og}_{half}_{oo}", tag="ev")
                            nc.scalar.activation(t[:], ps[oo][:], Copy)
                            # part writes ride the scalar ring: they pace with
                            # the evict copies, never blocking weight prefetch
                            if j == 0:
                                row = (og % 4) * 1024 + half * 512 + oo * 128
                                nc.scalar.dma_start(
                                    part0[og // 4][row : row + 128, :], t[:]
                                )
                            else:
                                row = og * 1024 + half * 512 + oo * 128
                                nc.scalar.dma_start(
                                    part1[row : row + 128, :], t[:]
                                )

            def ar0_half(a):
                nc.gpsimd.collective_compute(
                    "AllReduce", add, replica_groups=rg,
                    ins=[part0[a][:].opt()],
                    outs=[arout0[a][:].opt()],
                )

            def ar1_full():
                nc.gpsimd.collective_compute(
                    "AllReduce", add, replica_groups=rg,
                    ins=[part1[:].opt()],
                    outs=[arout1[:].opt()],
                )

            def dense_layer(k, j):
                # k in {2,3}; k==2 input = relu(AllReduce'd y1) applied
                # per-tile on the vector engine; k==3 input from gath2,
                # output to out_p
                wt = w2t_p if k == 2 else w3t_p
                ps = [
                    ppool.tile([128, NB], fp32, name=f"ps{k}_{j}_{o}", tag="ps")
                    for o in range(GT)
                ]
                for g in range(KT):
                    if k == 2:
                        raw = hpool.tile([128, NB], bf16, name=f"r{k}_{j}_{g}",
                                         tag="hin")
                        if j == 0:
                            hsrc = arout0[g // (KT // 2)]
                            row = (g % (KT // 2)) * 128
                        else:
                            hsrc = arout1
                            row = g * 128
                        nc.sync.dma_start(raw[:], hsrc[row : row + 128, :])
                        ht = hpool.tile([128, NB], bf16, name=f"h{k}_{j}_{g}",
                                        tag="hin")
                        nc.vector.tensor_scalar(
                            ht[:], raw[:], b1p[:, g : g + 1], 0.0, add, amax
                        )
                    else:
                        ht = hpool.tile([128, NB], bf16, name=f"h{k}_{j}_{g}",
                                        tag="hin")
                        nc.sync.dma_start(
                            ht[:], gath2[j][g * 128 : (g + 1) * 128, :]
                        )
                    wb = wpool.tile([128, GS], bf16, name=f"w{k}_{j}_{g}", tag="wblk")
                    nc.sync.dma_start(wb[:], wt[g * 128 : (g + 1) * 128, :])
                    for o in range(GT):
                        nc.tensor.matmul(
                            ps[o][:],
                            wb[:, o * 128 : (o + 1) * 128],
                            ht[:],
                            start=(g == 0),
                            stop=(g == KT - 1),
                        )
                for o in range(GT):
                    if k == 2:
                        ot = opool.tile(
                            [128, NB], bf16, name=f"o{k}_{j}_{o}", tag="hout"
                        )
                        nc.scalar.activation(
                            ot[:], ps[o][:], Relu, bias=sc[o][:, 6:7]
                        )
                        nc.scalar.dma_start(
                            slc2[j][o * 128 : (o + 1) * 128, :], ot[:]
                        )
                    else:
                        ot = opool.tile(
                            [128, NB], fp32, name=f"o{k}_{j}_{o}", tag="outp"
                        )
                        nc.scalar.activation(
                            ot[:], ps[o][:], Relu, bias=sc[o][:, 7:8]
                        )
                        nc.scalar.dma_start(
                            out_p[o * 128 : (o + 1) * 128, j * NB : (j + 1) * NB],
                            ot[:],
                        )

            # emission order = desired overlap order.  Preload L1c0's first
            # weight group ahead of the x stream so the PE starts ~12us in.
            pre = {0: load_w1_og(0, 0)}
            local_layer(0)
            l1rp(0, preloaded=pre)
            local_layer(1)
            ar0_half(0)
            ar0_half(1)
            l1rp(1)
            ar1_full()
            for j in range(NCHUNK):
                dense_layer(2, j)
                nc.gpsimd.collective_compute(
                    "AllGather", bypass, replica_groups=rg,
                    ins=[slc2[j][:].opt()],
                    outs=[gath2[j][:].opt()],
                )
            for j in range(NCHUNK):
                dense_layer(3, j)

    nc.compile()
    return nc


def _get_nc():
    if "nc" not in _compiled:
        _compiled["nc"] = _build_graph()
    return _compiled["nc"]


def kernel(x, W_local, b_local, W1, b1, W2, b2, W3, b3):
    from concourse.bass_utils import run_bass_kernel_spmd

    nc = _get_nc()

    x = np.asarray(x)
    W1p = np.asarray(W1)[_PERM_HALVES, :]      # rows = half-major outputs
    # b1 in half-major order as [128 rows-in-ktile, 64 ktiles]
    b1p = np.ascontiguousarray(
        np.asarray(b1)[_PERM_HALVES].reshape(KT, 128).T
    ).astype(np.float32)
    in_maps = []
    for r in range(N_CORES):
        sl = slice(r * GS, (r + 1) * GS)
        x_r = x[:, :, sl].transpose(0, 2, 1).astype(BF16)
        scal_r = np.concatenate(
            [
                np.asarray(W_local)[sl, :],
                np.asarray(b_local)[sl, None],
                np.asarray(b1)[sl, None],
                np.asarray(b2)[sl, None],
                np.asarray(b3)[sl, None],
            ],
            axis=1,
        ).astype(np.float32)
        in_maps.append(
            {
                "x": x_r,
                "scal": np.ascontiguousarray(scal_r),
                "b1p": b1p,
                # [own 1024 in-feats, 8192 half-major out-feats]
                "w1rp": np.ascontiguousarray(W1p[:, sl].T).astype(BF16),
                # [8192 half-major in-feats, own 1024 out-feats]
                "w2t": np.asarray(W2)[sl, :].T.astype(BF16)[_PERM_HALVES, :],
                "w3t": np.asarray(W3)[sl, :].T.astype(BF16),
            }
        )

    res = run_bass_kernel_spmd(nc, in_maps, core_ids=list(range(N_CORES)))

    out = np.empty((B, G), np.float32)
    for r in range(N_CORES):
        out[:, r * GS : (r + 1) * GS] = res.results[r]["out"].T
    return out


# revision 34
# speedup vs baseline: 1.2506x; 1.1293x over previous
"""Trainium2 Bass kernel for nn_AMLNeuralNetwork3D (dense_mlp).

Strategy: 8-way tensor parallel (column split on output features) for all
three 8192x8192 dense layers; the per-gene local layer shards along the
gene axis (matching the feature split).  After the local layer and after
L1/L2 the per-core feature slices are AllGather'd (concat on partition
axis = gene axis).  L3 slices are returned per-core and assembled on host.

Layout: activations are kept feature-major [features, batch] on chip so a
layer's output layout equals the next layer's input layout (contraction is
over the partition axis on the TensorEngine).  Weights are pre-transposed
on host to [in_features, out_slice] so all DMAs are wide/contiguous.

Compute in bf16 (full-rate on the PE, fp32 PSUM accumulation); measured
L2 rel-err of the full net in bf16 is ~5e-3.
"""

import sys

if "/opt/trn_rl_repo" not in sys.path:
    sys.path.insert(0, "/opt/trn_rl_repo")

import numpy as np
import ml_dtypes

N_CORES = 8
G = 8192          # genes / features
B = 1024          # batch
L = 4             # levels
GS = G // N_CORES # per-core feature slice (1024)
NB = 512          # batch chunk (one PSUM bank at fp32)
NCHUNK = B // NB  # 2
GT = GS // 128    # gene tiles per core slice (8)
KT = G // 128     # contraction tiles (64)

BF16 = ml_dtypes.bfloat16

_compiled = {}

# gathered-feature order when the first AllGather is split into two
# feature halves: [core0 f0:512, core1 f1024:1536, ...] then the second halves
_PERM_HALVES = np.concatenate(
    [np.arange(r * 1024, r * 1024 + 512) for r in range(8)]
    + [np.arange(r * 1024 + 512, (r + 1) * 1024) for r in range(8)]
)


def _build_graph():
    from concourse import bacc, tile
    from concourse.tile_rust import add_dep_helper
    import concourse.mybir as mybir

    fp32 = mybir.dt.float32
    bf16 = mybir.dt.bfloat16
    Relu = mybir.ActivationFunctionType.Relu
    mult = mybir.AluOpType.mult
    add = mybir.AluOpType.add
    bypass = mybir.AluOpType.bypass

    nc = bacc.Bacc(None, target_bir_lowering=False, num_devices=N_CORES)

    # ---- parameters (per-core shards; same graph on all cores) ----
    x_p = nc.declare_dram_parameter("x", [L, GS, B], bf16, isOutput=False)
    # per-feature scalars: cols 0..3 = W_local, 4 = b_local, 5..7 = b1..b3
    scal_p = nc.declare_dram_parameter("scal", [GS, 8], fp32, isOutput=False)
    w_p = [
        nc.declare_dram_parameter(f"w{k}t", [G, GS], bf16, isOutput=False)
        for k in (1, 2, 3)
    ]
    out_p = nc.declare_dram_parameter("out", [GS, B], fp32, isOutput=True)

    rg = [list(range(N_CORES))]

    with tile.TileContext(nc) as tc:
        with (
            tc.tile_pool(name="dram", bufs=1, space="DRAM") as dram,
            tc.tile_pool(name="scal", bufs=GT) as spool,
            tc.tile_pool(name="xin", bufs=12) as xpool,
            tc.tile_pool(name="loc", bufs=10) as lpool,
            tc.tile_pool(name="hin", bufs=28) as hpool,
            tc.tile_pool(name="wblk", bufs=16) as wpool,
            tc.tile_pool(name="hout", bufs=6) as opool,
            tc.tile_pool(name="psum", bufs=8, space="PSUM") as ppool,
        ):
            # bounce buffers for the 3 AllGather transitions x 2 chunks
            slc = [
                [
                    dram.tile([GS, NB], bf16, name=f"slc_{t}_{j}", tag=f"slc_{t}_{j}")
                    for j in range(NCHUNK)
                ]
                for t in range(3)
            ]
            _gath_space = "Shared"
            gath = [
                [
                    dram.tile(
                        [G, NB], bf16, name=f"gath_{t}_{j}", tag=f"gath_{t}_{j}",
                        addr_space=_gath_space,
                    )
                    for j in range(NCHUNK)
                ]
                for t in range(3)
            ]
            # transition-0 chunk-0 AllGather is split along the feature axis:
            # layer 1 starts accumulating K as soon as the first half lands.
            # (w1t rows are permuted on host to match the half-major order.)
            slc0h = [
                [
                    dram.tile(
                        [GS // 2, NB], bf16, name=f"slc0h{j}_{a}",
                        tag=f"slc0h{j}_{a}",
                    )
                    for a in range(2)
                ]
                for j in range(NCHUNK)
            ]
            gath0h = [
                [
                    dram.tile(
                        [G // 2, NB], bf16, name=f"gath0h{j}_{a}",
                        tag=f"gath0h{j}_{a}", addr_space=_gath_space,
                    )
                    for a in range(2)
                ]
                for j in range(NCHUNK)
            ]

            # --- PE warmup: the PE would otherwise idle until the first
            # gathered tiles arrive (~90us: launch barrier + the first
            # AllGather); dummy matmuls keep the HAM clock-gate warm
            # through the prologue at zero cost.
            wu_w = spool.tile([128, 128], bf16, name="wu_w", tag="wu_w")
            nc.sync.dma_start(wu_w[:], w_p[0][0:128, 0:128])
            wu_h = spool.tile([128, NB], bf16, name="wu_h", tag="wu_h")
            nc.sync.dma_start(wu_h[:], w_p[0][0:128, 0:NB])
            wu_ps = ppool.tile([128, NB], fp32, name="wu_ps", tag="ps")
            N_WARMUP = 290
            wu_gate = None
            for i in range(N_WARMUP):
                mi = nc.tensor.matmul(
                    wu_ps[:], wu_w[:], wu_h[:],
                    start=(i == 0), stop=(i == N_WARMUP - 1),
                )
                if i == 64:
                    wu_gate = mi
            wu_out = spool.tile([128, NB], bf16, name="wu_out", tag="wu_out")
            nc.scalar.activation(
                wu_out[:], wu_ps[:], mybir.ActivationFunctionType.Copy
            )
            wu_dram = dram.tile([128, NB], bf16, name="wu_dram", tag="wu_dram")
            nc.sync.dma_start(wu_dram[:], wu_out[:])

            # per-feature scalar tiles, persistent
            sc = []
            for gt in range(GT):
                s = spool.tile([128, 8], fp32, name=f"sc{gt}", tag="sc")
                nc.sync.dma_start(s[:], scal_p[gt * 128 : (gt + 1) * 128, :])
                sc.append(s)

            def local_layer(j):
                # returns the slc-write DMA instructions for optional gating
                slc_writes = []
                for gt in range(GT):
                    xt = []
                    for l in range(L):
                        t = xpool.tile([128, NB], bf16, name=f"x{j}_{gt}_{l}", tag="x")
                        nc.sync.dma_start(
                            t[:],
                            x_p[l, gt * 128 : (gt + 1) * 128, j * NB : (j + 1) * NB],
                        )
                        xt.append(t)
                    acc = lpool.tile([128, NB], bf16, name=f"a{j}_{gt}_0", tag="acc")
                    nc.vector.tensor_scalar(
                        acc[:], xt[0][:], sc[gt][:, 0:1], None, mult
                    )
                    for l in range(1, L):
                        acc2 = lpool.tile(
                            [128, NB], bf16, name=f"a{j}_{gt}_{l}", tag="acc"
                        )
                        nc.vector.scalar_tensor_tensor(
                            acc2[:], xt[l][:], sc[gt][:, l : l + 1], acc[:], mult, add
                        )
                        acc = acc2
                    h0 = opool.tile([128, NB], bf16, name=f"h0_{j}_{gt}", tag="hout")
                    nc.scalar.activation(h0[:], acc[:], Relu, bias=sc[gt][:, 4:5])
                    a, row = gt // 4, (gt % 4) * 128
                    # chunk-1 writes are gated on an L1 matmul marker; keep
                    # them off the sync ring so they can't head-of-line block
                    # L1's h-tile stream behind that gate
                    eng = nc.sync if j == 0 else nc.scalar
                    w = eng.dma_start(
                        slc0h[j][a][row : row + 128, :], h0[:]
                    )
                    slc_writes.append(w)
                    if j == 0 and gt == 3:
                        nc.gpsimd.collective_compute(
                            "AllGather", bypass, replica_groups=rg,
                            ins=[slc0h[0][0][:].opt()],
                            outs=[gath0h[0][0][:].opt()],
                        )
                return slc_writes

            def dense_layer(k, j):
                # k in {1,2,3}; input from gath[k-1][j]; output slice ->
                # slc[k][j] (k<3) or out_p (k==3)
                marker = {}
                src = gath[k - 1][j]
                wt = w_p[k - 1]
                halves = k == 1
                ps = [
                    ppool.tile([128, NB], fp32, name=f"ps{k}_{j}_{o}", tag="ps")
                    for o in range(GT)
                ]
                for g in range(KT):
                    ht = hpool.tile([128, NB], bf16, name=f"h{k}_{j}_{g}", tag="hin")
                    if halves:
                        hsrc = gath0h[j][g // (KT // 2)]
                        row = (g % (KT // 2)) * 128
                        nc.sync.dma_start(ht[:], hsrc[row : row + 128, :])
                    else:
                        nc.sync.dma_start(ht[:], src[g * 128 : (g + 1) * 128, :])
                    wb = wpool.tile([128, GS], bf16, name=f"w{k}_{j}_{g}", tag="wblk")
                    wdma = nc.sync.dma_start(wb[:], wt[g * 128 : (g + 1) * 128, :])
                    if k == 1 and j == 0 and g < 20:
                        # keep the W prefetch burst behind the local layer's
                        # x tiles on the sync ring
                        add_dep_helper(
                            getattr(wdma, "ins", wdma),
                            getattr(wu_gate, "ins", wu_gate),
                            reason="delay W prefetch past x tiles",
                        )
                    for o in range(GT):
                        mmi = nc.tensor.matmul(
                            ps[o][:],
                            wb[:, o * 128 : (o + 1) * 128],
                            ht[:],
                            start=(g == 0),
                            stop=(g == KT - 1),
                        )
                    marker[g] = mmi
                for o in range(GT):
                    if k < 3:
                        ot = opool.tile(
                            [128, NB], bf16, name=f"o{k}_{j}_{o}", tag="hout"
                        )
                        nc.scalar.activation(
                            ot[:], ps[o][:], Relu, bias=sc[o][:, 4 + k : 5 + k]
                        )
                        nc.sync.dma_start(
                            slc[k][j][o * 128 : (o + 1) * 128, :], ot[:]
                        )
                    else:
                        ot = opool.tile(
                            [128, NB], fp32, name=f"o{k}_{j}_{o}", tag="outp"
                        )
                        nc.scalar.activation(
                            ot[:], ps[o][:], Relu, bias=sc[o][:, 7:8]
                        )
                        nc.sync.dma_start(
                            out_p[o * 128 : (o + 1) * 128, j * NB : (j + 1) * NB],
                            ot[:],
                        )
                return marker

            def allgather(t, j):
                nc.gpsimd.collective_compute(
                    "AllGather",
                    bypass,
                    replica_groups=rg,
                    ins=[slc[t][j][:].opt()],
                    outs=[gath[t][j][:].opt()],
                )

            # emission order = desired overlap order
            local_layer(0)
            nc.gpsimd.collective_compute(
                "AllGather", bypass, replica_groups=rg,
                ins=[slc0h[0][1][:].opt()], outs=[gath0h[0][1][:].opt()],
            )
            slc1_writes = local_layer(1)
            for a in range(2):
                nc.gpsimd.collective_compute(
                    "AllGather", bypass, replica_groups=rg,
                    ins=[slc0h[1][a][:].opt()], outs=[gath0h[1][a][:].opt()],
                )
            first = True
            for k in (1, 2, 3):
                for j in range(NCHUNK):
                    marker = dense_layer(k, j)
                    if first:
                        # AG(0,c1)'s transfer otherwise collides with L1-c0's
                        # h/W DMA ramp-up right after the quarter-gathers;
                        # hold it until L1-c0 is ~1/3 done
                        gate = marker[4]
                        for w in slc1_writes:
                            add_dep_helper(
                                getattr(w, "ins", w),
                                getattr(gate, "ins", gate),
                                reason="defer AG(0,c1) past L1 ramp",
                            )
                        first = False
                    if k < 3:
                        allgather(k, j)

    nc.compile()
    return nc


def _get_nc():
    if "nc" not in _compiled:
        _compiled["nc"] = _build_graph()
    return _compiled["nc"]


def kernel(x, W_local, b_local, W1, b1, W2, b2, W3, b3):
    from concourse.bass_utils import run_bass_kernel_spmd

    nc = _get_nc()

    x = np.asarray(x)
    in_maps = []
    for r in range(N_CORES):
        sl = slice(r * GS, (r + 1) * GS)
        x_r = x[:, :, sl].transpose(0, 2, 1).astype(BF16)
        scal_r = np.concatenate(
            [
                np.asarray(W_local)[sl, :],
                np.asarray(b_local)[sl, None],
                np.asarray(b1)[sl, None],
                np.asarray(b2)[sl, None],
                np.asarray(b3)[sl, None],
            ],
            axis=1,
        ).astype(np.float32)
        in_maps.append(
            {
                "x": x_r,
                "scal": np.ascontiguousarray(scal_r),
                "w1t": np.asarray(W1)[sl, :].T.astype(BF16)[_PERM_HALVES, :],
                "w2t": np.asarray(W2)[sl, :].T.astype(BF16),
                "w3t": np.asarray(W3)[sl, :].T.astype(BF16),
            }
        )

    res = run_bass_kernel_spmd(nc, in_maps, core_ids=list(range(N_CORES)))

    out = np.empty((B, G), np.float32)
    for r in range(N_CORES):
        out[:, r * GS : (r + 1) * GS] = res.results[r]["out"].T
    return out

